# revision 40
# baseline (speedup 1.0000x reference)
"""MoE top-2 routing kernel for 8 Trainium2 NeuronCores (expert parallelism).

Problem (hardcoded): x[4,2048,1024] fp32, gate_w[1024,8], w1[8,1024,4096],
b1[8,4096], w2[8,4096,1024], b2[8,1024].  TOP_K=2.

Strategy:
  - Host: gate softmax + top-2 + renormalize (tiny: 8192x1024x8 matmul),
    dispatch tokens to experts (this IS the sharding: core e owns expert e).
  - Device (SPMD, one program, 8 cores): core e computes, for its padded
    token batch X_e [C,1024]:  y = combine_w * (gelu(X_e @ W1_e + b1_e) @ W2_e + b2_e)
    in bf16 with fp32 PSUM accumulation.
  - Host: scatter-add the two expert contributions per token; return
    (out [4,2048,1024] fp32, routing_weights [8192,8] fp32).

Zero-weight experts are mathematically exact to skip: the reference's dense
combine multiplies non-selected experts by an exact 0.
"""

import time

import numpy as np
import ml_dtypes

import concourse.bacc as bacc
import concourse.tile as tile
import concourse.mybir as mybir
from concourse.bass_utils import run_bass_kernel_spmd

BF16 = mybir.dt.bfloat16
F32 = mybir.dt.float32

E = 8          # experts == cores
TOP_K = 2
H = 1024
F = 4096
T = 8192
KT = H // 128  # 8 contraction tiles for MM1
FT = F // 128  # 32 f tiles

# module-level cache: C -> compiled Bacc program (jax/PJRT caches the NEFF)
_PROGRAM_CACHE: dict[int, object] = {}

# test.py reads this for exec_time_ns after a traced run
LAST_RESULTS = None


def _slices(C, C_x=None):
    """Token column slices over the exact token count C_x: near-equal even
    widths <=512.

    Near-equal widths keep every slice's matmul free-dim large enough that
    LDWEIGHTS (~97ns) hides behind the matmul stream, and keep the per-slice
    W1 streaming demand (8MB per slice regardless of width) safely below
    HBM bandwidth."""
    if C_x is None:
        C_x = C
    if C_x <= 512:
        return [(0, C_x)]
    # slice 0 at full width: at startup the PE consumes W1 chunks while the
    # DMA queues are still ramping — more matmul time per chunk hides that
    rest = C_x - 512
    n = -(-rest // 512)
    half_units = rest // 2             # widths kept even (4-byte bf16 pairs)
    widths = [2 * (half_units // n)] * n
    for i in range(half_units - (half_units // n) * n):
        widths[i] += 2
    out = [(0, 512)]
    off = 512
    for w in widths:
        out.append((off, w))
        off += w
    return out


def build_moe_program(C, C_x=None):
    """One SPMD program: FFN for one expert over C padded tokens (MM1
    trimmed to the exact C_x)."""
    NT = C // 128
    nc = bacc.Bacc(
        "TRN2", target_bir_lowering=False, debug=False,
        enable_asserts=False, num_devices=E,
    )
    xt_d = nc.dram_tensor("xt", [H, C], BF16, kind="ExternalInput").ap()
    w1c_d = nc.dram_tensor("w1c", [F, 1024], BF16, kind="ExternalInput").ap()
    w2_d = nc.dram_tensor("w2", [F, H], BF16, kind="ExternalInput").ap()
    b1t_d = nc.dram_tensor("b1t", [128, FT], F32, kind="ExternalInput").ap()
    b2t_d = nc.dram_tensor("b2t", [128, H // 128], F32, kind="ExternalInput").ap()
    wb_d = nc.dram_tensor("wb", [128, C], F32, kind="ExternalInput").ap()
    yt_d = nc.dram_tensor("yt", [H, C], F32, kind="ExternalOutput").ap()

    with tile.TileContext(nc) as tc:
        with (
            tc.tile_pool(name="resid", bufs=1) as resid,
            tc.tile_pool(name="w1s", bufs=6) as w1s,
            tc.tile_pool(name="gt", bufs=2) as gtp,
            tc.tile_pool(name="yst", bufs=3) as yst,
            tc.tile_pool(name="ps", bufs=4, space="PSUM") as psp,
        ):
            # ---- resident tiles; DMA emission is ordered so the PE can
            # start slice 0 ASAP: xt[slice0] + b1t first, w1 chunks stream
            # inside the slice loop, w2/b2b/wt arrive during MM1(slice0) ----
            slices = _slices(C, C_x)
            # the first weight chunks gate the earliest matmuls — queue them
            # ahead of everything (DMA queues ramp for the first ~10us)
            N_HOIST = 4
            w1t_early = []
            for f in range(N_HOIST):
                w1t_e = w1s.tile([128, 1024], BF16, name=f"w1t_early{f}",
                                 tag="w1t")
                nc.sync.dma_start(w1t_e[:], w1c_d[f * 128:(f + 1) * 128, :])
                w1t_early.append(w1t_e)
            xt_t = [resid.tile([128, C], BF16, name=f"xt{k}", tag=f"xt{k}")
                    for k in range(KT)]
            s0_off, s0_len = slices[0]
            # slice 0's xt in two column halves: matmul #1 waits on 0.5MB
            s0_halves = ([(s0_off, s0_len // 2),
                          (s0_off + s0_len // 2, s0_len - s0_len // 2)]
                         if s0_len >= 512 else [(s0_off, s0_len)])
            for h_off, h_len in s0_halves:
                for k in range(KT):
                    nc.sync.dma_start(xt_t[k][:, h_off:h_off + h_len],
                                      xt_d[k * 128:(k + 1) * 128, h_off:h_off + h_len])
            b1t = resid.tile([128, FT], F32, name="b1t", tag="b1t")
            nc.sync.dma_start(b1t[:], b1t_d[:])
            w2_t = [resid.tile([128, H], BF16, name=f"w2_{f}", tag=f"w2_{f}")
                    for f in range(FT)]
            b2t = resid.tile([128, H // 128], F32, name="b2t", tag="b2t")
            wb = resid.tile([128, C], F32, name="wb", tag="wb")

            for si, (s_off, s_len) in enumerate(slices):
                # ---- MM1 + gelu: gt[f] = gelu(b1 + sum_k W1[k,f].T @ XT[k, slice]) ----
                # slice 0 runs MM1 in two column halves sharing each w1 chunk:
                # the first matmul is gated by half the xt bytes, while the
                # W1 streaming rate (one 256KB chunk per f) is unchanged
                subs = (s0_halves if si == 0 else [(s_off, s_len)])
                gts = []
                for f in range(FT):
                    if si == 0 and f < N_HOIST:
                        w1t = w1t_early[f]
                    else:
                        w1t = w1s.tile([128, 1024], BF16,
                                       name=f"w1t_s{s_off}_f{f}", tag="w1t")
                        nc.sync.dma_start(w1t[:], w1c_d[f * 128:(f + 1) * 128, :])
                    gt = gtp.tile([128, 512], BF16, name=f"gt_{s_off}_{f}",
                                  tag=f"gt{f}")
                    for b_off, b_len in subs:
                        ps1 = psp.tile([128, b_len], F32,
                                       name=f"ps1_{b_off}_{f}", tag="ps1")
                        for k in range(KT):
                            nc.tensor.matmul(
                                ps1[:],
                                w1t[:, k * 128:(k + 1) * 128],
                                xt_t[k][:, b_off:b_off + b_len],
                                start=(k == 0), stop=(k == KT - 1),
                            )
                        g_off = b_off - s_off
                        nc.scalar.activation(
                            gt[:, g_off:g_off + b_len], ps1[:],
                            mybir.ActivationFunctionType.Gelu,
                            bias=b1t[:, f:f + 1],
                        )
                    gts.append(gt)

                # prefetch next slice's xt columns while this slice computes
                # (emitted after MM1's w1 chunks so it doesn't queue ahead of
                # the critical first weight chunk at startup)
                if si + 1 < len(slices):
                    n_off, n_len = slices[si + 1]
                    for k in range(KT):
                        nc.sync.dma_start(
                            xt_t[k][:, n_off:n_off + n_len],
                            xt_d[k * 128:(k + 1) * 128, n_off:n_off + n_len])

                if si == 0:
                    # w2/b2t/wb fetched while the PE runs MM1(slice 0)
                    for f in range(FT):
                        nc.sync.dma_start(w2_t[f][:],
                                          w2_d[f * 128:(f + 1) * 128, :])
                    nc.sync.dma_start(b2t[:], b2t_d[:])
                    nc.sync.dma_start(wb[:], wb_d[:])

                # ---- MM2 (transposed out: YT[h, tokens], exact token width)
                #      + bias + combine-weight scale + store.
                # Early slices: h-groups of 4 with f outer, so w2[f] tiles
                # are consumed progressively while the 8MB w2 stream is still
                # in flight (4 PSUM banks). Final slice: h outer with f inner,
                # so only one drain chain trails the last matmul ----
                last = si == len(slices) - 1
                hg_size = 1 if last else 4
                for hg in range(0, H // 128, hg_size):
                    pss = [psp.tile([128, s_len], F32,
                                    name=f"ps2_{s_off}_{hg + j}", tag="ps2")
                           for j in range(hg_size)]
                    for f in range(FT):
                        for j in range(hg_size):
                            h = hg + j
                            nc.tensor.matmul(
                                pss[j][:],
                                w2_t[f][:, h * 128:(h + 1) * 128],
                                gts[f][:, :s_len],
                                start=(f == 0), stop=(f == FT - 1),
                            )
                    for j in range(hg_size):
                        h = hg + j
                        y_t = yst.tile([128, 512], F32, name=f"y_{s_off}_{h}",
                                       tag="y")
                        nc.vector.tensor_scalar_add(y_t[:, :s_len], pss[j][:],
                                                    b2t[:, h:h + 1])
                        nc.vector.tensor_mul(y_t[:, :s_len], y_t[:, :s_len],
                                             wb[:, s_off:s_off + s_len])
                        nc.sync.dma_start(
                            yt_d[h * 128:(h + 1) * 128, s_off:s_off + s_len],
                            y_t[:, :s_len])

    nc.compile()
    return nc


def kernel(x, gate_w, w1, b1, w2, b2):
    global LAST_RESULTS
    x = np.asarray(x, np.float32)
    gate_w = np.asarray(gate_w, np.float32)
    w1 = np.asarray(w1, np.float32)
    b1 = np.asarray(b1, np.float32)
    w2 = np.asarray(w2, np.float32)
    b2 = np.asarray(b2, np.float32)

    B, S, _ = x.shape
    xt = x.reshape(T, H)

    # ---- gate: softmax over experts, top-2, renormalize (host, fp32) ----
    logits = xt @ gate_w                                   # [T, E]
    logits = logits - logits.max(axis=1, keepdims=True)
    ex = np.exp(logits)
    rw = ex / ex.sum(axis=1, keepdims=True)                # routing_weights
    sel = np.argsort(-rw, axis=1, kind="stable")[:, :TOP_K]  # ties: low idx first
    tw = np.take_along_axis(rw, sel, axis=1)
    tw = tw / tw.sum(axis=1, keepdims=True)

    # ---- dispatch: token lists per expert ----
    idx_e, w_e = [], []
    for e in range(E):
        m0 = sel[:, 0] == e
        m1 = sel[:, 1] == e
        idx_e.append(np.concatenate([np.nonzero(m0)[0], np.nonzero(m1)[0]]))
        w_e.append(np.concatenate([tw[m0, 0], tw[m1, 1]]).astype(np.float32))
    # SBUF budget caps the per-core token capacity; if routing is ever
    # imbalanced enough to exceed it, split dispatch into multiple rounds
    C_SAFE = 2304
    rounds = max(1, -(-max(len(i) for i in idx_e) // C_SAFE))

    # per-expert weight tensors are round-invariant
    wt_maps = []
    for e in range(E):
        # w1c[f*128+p, k*128+j] = w1[e, k*128+p, f*128+j]
        w1c = np.ascontiguousarray(
            w1[e].astype(ml_dtypes.bfloat16)
            .reshape(KT, 128, FT, 128).transpose(2, 1, 0, 3).reshape(F, 1024)
        )
        wt_maps.append({
            "w1c": w1c,
            "w2": np.ascontiguousarray(w2[e].astype(ml_dtypes.bfloat16)),
            "b1t": np.ascontiguousarray(b1[e].reshape(FT, 128).T.astype(np.float32)),
            "b2t": np.ascontiguousarray(b2[e].reshape(H // 128, 128).T.astype(np.float32)),
        })

    xt_bf = xt.astype(ml_dtypes.bfloat16)
    out = np.zeros((T, H), np.float32)
    for r in range(rounds):
        idx_r = [i[r::rounds] for i in idx_e]
        w_r = [w[r::rounds] for w in w_e]
        counts = [len(i) for i in idx_r]
        C = max(512, -(-max(counts) // 128) * 128)
        C_x = min(C, -(-max(counts) // 2) * 2)   # exact MM1 width, 2-aligned

        key = (C, C_x)
        nc = _PROGRAM_CACHE.get(key)
        if nc is None:
            nc = build_moe_program(C, C_x)
            _PROGRAM_CACHE[key] = nc

        in_maps = []
        for e in range(E):
            n = counts[e]
            xe = np.zeros((C, H), ml_dtypes.bfloat16)
            xe[:n] = xt_bf[idx_r[e]]
            wpad = np.zeros(C, np.float32)
            wpad[:n] = w_r[e]
            in_maps.append({
                "xt": np.ascontiguousarray(xe.T),
                "wb": np.ascontiguousarray(np.broadcast_to(wpad, (128, C))),
                **wt_maps[e],
            })

        res = None
        for attempt in range(3):
            try:
                res = run_bass_kernel_spmd(nc, in_maps, core_ids=list(range(E)))
                break
            except Exception:
                # transient NRT_EXEC_UNIT_UNRECOVERABLE has been observed on
                # a fresh device; retry before giving up
                if attempt == 2:
                    raise
                time.sleep(2.0)
        LAST_RESULTS = res

        # ---- combine: each token gets exactly two expert contributions ----
        for e in range(E):
            n = counts[e]
            if n:
                out[idx_r[e]] += res.results[e]["yt"][:, :n].T

    return out.reshape(B, S, H), rw


# revision 42
# speedup vs baseline: 1.0118x; 1.0118x over previous
"""MoE top-2 routing kernel for 8 Trainium2 NeuronCores (expert parallelism).

Problem (hardcoded): x[4,2048,1024] fp32, gate_w[1024,8], w1[8,1024,4096],
b1[8,4096], w2[8,4096,1024], b2[8,1024].  TOP_K=2.

Strategy:
  - Host: gate softmax + top-2 + renormalize (tiny: 8192x1024x8 matmul),
    dispatch tokens to experts (this IS the sharding: core e owns expert e).
  - Device (SPMD, one program, 8 cores): core e computes, for its padded
    token batch X_e [C,1024]:  y = combine_w * (gelu(X_e @ W1_e + b1_e) @ W2_e + b2_e)
    in bf16 with fp32 PSUM accumulation.
  - Host: scatter-add the two expert contributions per token; return
    (out [4,2048,1024] fp32, routing_weights [8192,8] fp32).

Zero-weight experts are mathematically exact to skip: the reference's dense
combine multiplies non-selected experts by an exact 0.
"""

import time

import numpy as np
import ml_dtypes

import concourse.bacc as bacc
import concourse.tile as tile
import concourse.mybir as mybir
from concourse.bass_utils import run_bass_kernel_spmd

BF16 = mybir.dt.bfloat16
F32 = mybir.dt.float32

E = 8          # experts == cores
TOP_K = 2
H = 1024
F = 4096
T = 8192
KT = H // 128  # 8 contraction tiles for MM1
FT = F // 128  # 32 f tiles

# module-level cache: C -> compiled Bacc program (jax/PJRT caches the NEFF)
_PROGRAM_CACHE: dict[int, object] = {}

# test.py reads this for exec_time_ns after a traced run
LAST_RESULTS = None


def _slices(C, C_x=None):
    """Token column slices over the exact token count C_x: near-equal even
    widths <=512.

    Near-equal widths keep every slice's matmul free-dim large enough that
    LDWEIGHTS (~97ns) hides behind the matmul stream, and keep the per-slice
    W1 streaming demand (8MB per slice regardless of width) safely below
    HBM bandwidth."""
    if C_x is None:
        C_x = C
    if C_x <= 512:
        return [(0, C_x)]
    # slice 0 at full width: at startup the PE consumes W1 chunks while the
    # DMA queues are still ramping — more matmul time per chunk hides that
    rest = C_x - 512
    n = -(-rest // 512)
    half_units = rest // 2             # widths kept even (4-byte bf16 pairs)
    widths = [2 * (half_units // n)] * n
    for i in range(half_units - (half_units // n) * n):
        widths[i] += 2
    out = [(0, 512)]
    off = 512
    for w in widths:
        out.append((off, w))
        off += w
    return out


def build_moe_program(C, C_x=None):
    """One SPMD program: FFN for one expert over C padded tokens (MM1
    trimmed to the exact C_x)."""
    NT = C // 128
    nc = bacc.Bacc(
        "TRN2", target_bir_lowering=False, debug=False,
        enable_asserts=False, num_devices=E,
    )
    xt_d = nc.dram_tensor("xt", [H, C], BF16, kind="ExternalInput").ap()
    w1c_d = nc.dram_tensor("w1c", [F, 1024], BF16, kind="ExternalInput").ap()
    w2_d = nc.dram_tensor("w2", [F, H], BF16, kind="ExternalInput").ap()
    b1t_d = nc.dram_tensor("b1t", [128, FT], F32, kind="ExternalInput").ap()
    b2t_d = nc.dram_tensor("b2t", [128, H // 128], F32, kind="ExternalInput").ap()
    wb_d = nc.dram_tensor("wb", [128, C], F32, kind="ExternalInput").ap()
    yt_d = nc.dram_tensor("yt", [H, C], F32, kind="ExternalOutput").ap()

    with tile.TileContext(nc) as tc:
        with (
            tc.tile_pool(name="resid", bufs=1) as resid,
            tc.tile_pool(name="w1s", bufs=6) as w1s,
            tc.tile_pool(name="gt", bufs=2) as gtp,
            tc.tile_pool(name="yst", bufs=3) as yst,
            tc.tile_pool(name="ps", bufs=4, space="PSUM") as psp,
        ):
            # ---- resident tiles; DMA emission is ordered so the PE can
            # start slice 0 ASAP: xt[slice0] + b1t first, w1 chunks stream
            # inside the slice loop, w2/b2b/wt arrive during MM1(slice0) ----
            slices = _slices(C, C_x)
            # the first weight chunks gate the earliest matmuls — queue them
            # ahead of everything (DMA queues ramp for the first ~10us)
            N_HOIST = 4
            w1t_early = []
            for f in range(N_HOIST):
                w1t_e = w1s.tile([128, 1024], BF16, name=f"w1t_early{f}",
                                 tag="w1t")
                nc.sync.dma_start(w1t_e[:], w1c_d[f * 128:(f + 1) * 128, :])
                w1t_early.append(w1t_e)
            xt_t = [resid.tile([128, C], BF16, name=f"xt{k}", tag=f"xt{k}")
                    for k in range(KT)]
            s0_off, s0_len = slices[0]
            for k in range(KT):
                nc.sync.dma_start(xt_t[k][:, s0_off:s0_off + s0_len],
                                  xt_d[k * 128:(k + 1) * 128, s0_off:s0_off + s0_len])
            b1t = resid.tile([128, FT], F32, name="b1t", tag="b1t")
            nc.sync.dma_start(b1t[:], b1t_d[:])
            w2_t = [resid.tile([128, H], BF16, name=f"w2_{f}", tag=f"w2_{f}")
                    for f in range(FT)]
            b2t = resid.tile([128, H // 128], F32, name="b2t", tag="b2t")
            wb = resid.tile([128, C], F32, name="wb", tag="wb")

            for si, (s_off, s_len) in enumerate(slices):
                # ---- MM1 + gelu: gt[f] = gelu(b1 + sum_k W1[k,f].T @ XT[k, slice]) ----
                gts = []
                for f in range(FT):
                    if si == 0 and f < N_HOIST:
                        w1t = w1t_early[f]
                    else:
                        w1t = w1s.tile([128, 1024], BF16,
                                       name=f"w1t_s{s_off}_f{f}", tag="w1t")
                        nc.sync.dma_start(w1t[:], w1c_d[f * 128:(f + 1) * 128, :])
                    ps1 = psp.tile([128, s_len], F32, name=f"ps1_{s_off}_{f}",
                                   tag="ps1")
                    for k in range(KT):
                        nc.tensor.matmul(
                            ps1[:],
                            w1t[:, k * 128:(k + 1) * 128],
                            xt_t[k][:, s_off:s_off + s_len],
                            start=(k == 0), stop=(k == KT - 1),
                        )
                    gt = gtp.tile([128, 512], BF16, name=f"gt_{s_off}_{f}",
                                  tag=f"gt{f}")
                    nc.scalar.activation(
                        gt[:, :s_len], ps1[:],
                        mybir.ActivationFunctionType.Gelu,
                        bias=b1t[:, f:f + 1],
                    )
                    gts.append(gt)

                # prefetch next slice's xt columns while this slice computes
                # (emitted after MM1's w1 chunks so it doesn't queue ahead of
                # the critical first weight chunk at startup)
                if si + 1 < len(slices):
                    n_off, n_len = slices[si + 1]
                    for k in range(KT):
                        nc.sync.dma_start(
                            xt_t[k][:, n_off:n_off + n_len],
                            xt_d[k * 128:(k + 1) * 128, n_off:n_off + n_len])

                if si == 0:
                    # w2/b2t/wb fetched while the PE runs MM1(slice 0)
                    for f in range(FT):
                        nc.sync.dma_start(w2_t[f][:],
                                          w2_d[f * 128:(f + 1) * 128, :])
                    nc.sync.dma_start(b2t[:], b2t_d[:])
                    nc.sync.dma_start(wb[:], wb_d[:])

                # ---- MM2 (transposed out: YT[h, tokens], exact token width)
                #      + bias + combine-weight scale + store.
                # Early slices: h-groups of 4 with f outer, so w2[f] tiles
                # are consumed progressively while the 8MB w2 stream is still
                # in flight (4 PSUM banks). Final slice: h outer with f inner,
                # so only one drain chain trails the last matmul ----
                last = si == len(slices) - 1
                hg_size = 1 if last else 4
                for hg in range(0, H // 128, hg_size):
                    pss = [psp.tile([128, s_len], F32,
                                    name=f"ps2_{s_off}_{hg + j}", tag="ps2")
                           for j in range(hg_size)]
                    for f in range(FT):
                        for j in range(hg_size):
                            h = hg + j
                            nc.tensor.matmul(
                                pss[j][:],
                                w2_t[f][:, h * 128:(h + 1) * 128],
                                gts[f][:, :s_len],
                                start=(f == 0), stop=(f == FT - 1),
                            )
                    for j in range(hg_size):
                        h = hg + j
                        y_t = yst.tile([128, 512], F32, name=f"y_{s_off}_{h}",
                                       tag="y")
                        nc.vector.tensor_scalar_add(y_t[:, :s_len], pss[j][:],
                                                    b2t[:, h:h + 1])
                        nc.vector.tensor_mul(y_t[:, :s_len], y_t[:, :s_len],
                                             wb[:, s_off:s_off + s_len])
                        nc.sync.dma_start(
                            yt_d[h * 128:(h + 1) * 128, s_off:s_off + s_len],
                            y_t[:, :s_len])

    nc.compile()
    return nc


def kernel(x, gate_w, w1, b1, w2, b2):
    global LAST_RESULTS
    x = np.asarray(x, np.float32)
    gate_w = np.asarray(gate_w, np.float32)
    w1 = np.asarray(w1, np.float32)
    b1 = np.asarray(b1, np.float32)
    w2 = np.asarray(w2, np.float32)
    b2 = np.asarray(b2, np.float32)

    B, S, _ = x.shape
    xt = x.reshape(T, H)

    # ---- gate: softmax over experts, top-2, renormalize (host, fp32) ----
    logits = xt @ gate_w                                   # [T, E]
    logits = logits - logits.max(axis=1, keepdims=True)
    ex = np.exp(logits)
    rw = ex / ex.sum(axis=1, keepdims=True)                # routing_weights
    sel = np.argsort(-rw, axis=1, kind="stable")[:, :TOP_K]  # ties: low idx first
    tw = np.take_along_axis(rw, sel, axis=1)
    tw = tw / tw.sum(axis=1, keepdims=True)

    # ---- dispatch: token lists per expert ----
    idx_e, w_e = [], []
    for e in range(E):
        m0 = sel[:, 0] == e
        m1 = sel[:, 1] == e
        idx_e.append(np.concatenate([np.nonzero(m0)[0], np.nonzero(m1)[0]]))
        w_e.append(np.concatenate([tw[m0, 0], tw[m1, 1]]).astype(np.float32))
    # SBUF budget caps the per-core token capacity; if routing is ever
    # imbalanced enough to exceed it, split dispatch into multiple rounds
    C_SAFE = 2304
    rounds = max(1, -(-max(len(i) for i in idx_e) // C_SAFE))

    # per-expert weight tensors are round-invariant
    wt_maps = []
    for e in range(E):
        # w1c[f*128+p, k*128+j] = w1[e, k*128+p, f*128+j]
        w1c = np.ascontiguousarray(
            w1[e].astype(ml_dtypes.bfloat16)
            .reshape(KT, 128, FT, 128).transpose(2, 1, 0, 3).reshape(F, 1024)
        )
        wt_maps.append({
            "w1c": w1c,
            "w2": np.ascontiguousarray(w2[e].astype(ml_dtypes.bfloat16)),
            "b1t": np.ascontiguousarray(b1[e].reshape(FT, 128).T.astype(np.float32)),
            "b2t": np.ascontiguousarray(b2[e].reshape(H // 128, 128).T.astype(np.float32)),
        })

    xt_bf = xt.astype(ml_dtypes.bfloat16)
    out = np.zeros((T, H), np.float32)
    for r in range(rounds):
        idx_r = [i[r::rounds] for i in idx_e]
        w_r = [w[r::rounds] for w in w_e]
        counts = [len(i) for i in idx_r]
        C = max(512, -(-max(counts) // 128) * 128)
        C_x = min(C, -(-max(counts) // 2) * 2)   # exact MM1 width, 2-aligned

        key = (C, C_x)
        nc = _PROGRAM_CACHE.get(key)
        if nc is None:
            nc = build_moe_program(C, C_x)
            _PROGRAM_CACHE[key] = nc

        in_maps = []
        for e in range(E):
            n = counts[e]
            xe = np.zeros((C, H), ml_dtypes.bfloat16)
            xe[:n] = xt_bf[idx_r[e]]
            wpad = np.zeros(C, np.float32)
            wpad[:n] = w_r[e]
            in_maps.append({
                "xt": np.ascontiguousarray(xe.T),
                "wb": np.ascontiguousarray(np.broadcast_to(wpad, (128, C))),
                **wt_maps[e],
            })

        res = None
        for attempt in range(3):
            try:
                res = run_bass_kernel_spmd(nc, in_maps, core_ids=list(range(E)))
                break
            except Exception:
                # transient NRT_EXEC_UNIT_UNRECOVERABLE has been observed on
                # a fresh device; retry before giving up
                if attempt == 2:
                    raise
                time.sleep(2.0)
        LAST_RESULTS = res

        # ---- combine: each token gets exactly two expert contributions ----
        for e in range(E):
            n = counts[e]
            if n:
                out[idx_r[e]] += res.results[e]["yt"][:, :n].T

    return out.reshape(B, S, H), rw


# revision 44
# speedup vs baseline: 1.0142x; 1.0024x over previous
"""MoE top-2 routing kernel for 8 Trainium2 NeuronCores (expert parallelism).

Problem (hardcoded): x[4,2048,1024] fp32, gate_w[1024,8], w1[8,1024,4096],
b1[8,4096], w2[8,4096,1024], b2[8,1024].  TOP_K=2.

Strategy:
  - Host: gate softmax + top-2 + renormalize (tiny: 8192x1024x8 matmul),
    dispatch tokens to experts (this IS the sharding: core e owns expert e).
  - Device (SPMD, one program, 8 cores): core e computes, for its padded
    token batch X_e [C,1024]:  y = combine_w * (gelu(X_e @ W1_e + b1_e) @ W2_e + b2_e)
    in bf16 with fp32 PSUM accumulation.
  - Host: scatter-add the two expert contributions per token; return
    (out [4,2048,1024] fp32, routing_weights [8192,8] fp32).

Zero-weight experts are mathematically exact to skip: the reference's dense
combine multiplies non-selected experts by an exact 0.
"""

import time

import numpy as np
import ml_dtypes

import concourse.bacc as bacc
import concourse.tile as tile
import concourse.mybir as mybir
from concourse.bass_utils import run_bass_kernel_spmd

BF16 = mybir.dt.bfloat16
F32 = mybir.dt.float32

E = 8          # experts == cores
TOP_K = 2
H = 1024
F = 4096
T = 8192
KT = H // 128  # 8 contraction tiles for MM1
FT = F // 128  # 32 f tiles

# module-level cache: C -> compiled Bacc program (jax/PJRT caches the NEFF)
_PROGRAM_CACHE: dict[int, object] = {}

# test.py reads this for exec_time_ns after a traced run
LAST_RESULTS = None


def _slices(C, C_x=None):
    """Token column slices over the exact token count C_x: near-equal even
    widths <=512.

    Near-equal widths keep every slice's matmul free-dim large enough that
    LDWEIGHTS (~97ns) hides behind the matmul stream, and keep the per-slice
    W1 streaming demand (8MB per slice regardless of width) safely below
    HBM bandwidth."""
    if C_x is None:
        C_x = C
    if C_x <= 512:
        return [(0, C_x)]
    # slice 0 at full width: at startup the PE consumes W1 chunks while the
    # DMA queues are still ramping — more matmul time per chunk hides that
    rest = C_x - 512
    n = -(-rest // 512)
    half_units = rest // 2             # widths kept even (4-byte bf16 pairs)
    widths = [2 * (half_units // n)] * n
    for i in range(half_units - (half_units // n) * n):
        widths[i] += 2
    out = [(0, 512)]
    off = 512
    for w in widths:
        out.append((off, w))
        off += w
    return out


def build_moe_program(C, C_x=None):
    """One SPMD program: FFN for one expert over C padded tokens (MM1
    trimmed to the exact C_x)."""
    NT = C // 128
    nc = bacc.Bacc(
        "TRN2", target_bir_lowering=False, debug=False,
        enable_asserts=False, num_devices=E,
    )
    xt_d = nc.dram_tensor("xt", [H, C], BF16, kind="ExternalInput").ap()
    w1c_d = nc.dram_tensor("w1c", [F, 1024], BF16, kind="ExternalInput").ap()
    w2_d = nc.dram_tensor("w2", [F, H], BF16, kind="ExternalInput").ap()
    b1t_d = nc.dram_tensor("b1t", [128, FT], F32, kind="ExternalInput").ap()
    b2t_d = nc.dram_tensor("b2t", [128, H // 128], F32, kind="ExternalInput").ap()
    wb_d = nc.dram_tensor("wb", [128, C], F32, kind="ExternalInput").ap()
    yt_d = nc.dram_tensor("yt", [H, C], F32, kind="ExternalOutput").ap()

    with tile.TileContext(nc) as tc:
        with (
            tc.tile_pool(name="resid", bufs=1) as resid,
            tc.tile_pool(name="w1s", bufs=6) as w1s,
            tc.tile_pool(name="gt", bufs=2) as gtp,
            tc.tile_pool(name="yst", bufs=3) as yst,
            tc.tile_pool(name="ps", bufs=4, space="PSUM") as psp,
            # ps1: MM1 drains via gelu within ~0.4us — 3 banks suffice.
            # ps2: 5 banks so an h-group's first start=True matmul never
            # waits on the previous group's still-draining bank.
        ):
            # ---- resident tiles; DMA emission is ordered so the PE can
            # start slice 0 ASAP: xt[slice0] + b1t first, w1 chunks stream
            # inside the slice loop, w2/b2b/wt arrive during MM1(slice0) ----
            slices = _slices(C, C_x)
            # the first weight chunks gate the earliest matmuls — queue them
            # ahead of everything (DMA queues ramp for the first ~10us)
            N_HOIST = 4
            w1t_early = []
            for f in range(N_HOIST):
                w1t_e = w1s.tile([128, 1024], BF16, name=f"w1t_early{f}",
                                 tag="w1t")
                nc.sync.dma_start(w1t_e[:], w1c_d[f * 128:(f + 1) * 128, :])
                w1t_early.append(w1t_e)
            xt_t = [resid.tile([128, C], BF16, name=f"xt{k}", tag=f"xt{k}")
                    for k in range(KT)]
            s0_off, s0_len = slices[0]
            for k in range(KT):
                nc.sync.dma_start(xt_t[k][:, s0_off:s0_off + s0_len],
                                  xt_d[k * 128:(k + 1) * 128, s0_off:s0_off + s0_len])
            b1t = resid.tile([128, FT], F32, name="b1t", tag="b1t")
            nc.sync.dma_start(b1t[:], b1t_d[:])
            w2_t = [resid.tile([128, H], BF16, name=f"w2_{f}", tag=f"w2_{f}")
                    for f in range(FT)]
            b2t = resid.tile([128, H // 128], F32, name="b2t", tag="b2t")
            wb = resid.tile([128, C], F32, name="wb", tag="wb")

            for si, (s_off, s_len) in enumerate(slices):
                # ---- MM1 + gelu: gt[f] = gelu(b1 + sum_k W1[k,f].T @ XT[k, slice]) ----
                gts = []
                for f in range(FT):
                    if si == 0 and f < N_HOIST:
                        w1t = w1t_early[f]
                    else:
                        w1t = w1s.tile([128, 1024], BF16,
                                       name=f"w1t_s{s_off}_f{f}", tag="w1t")
                        nc.sync.dma_start(w1t[:], w1c_d[f * 128:(f + 1) * 128, :])
                    ps1 = psp.tile([128, s_len], F32, name=f"ps1_{s_off}_{f}",
                                   tag="ps1", bufs=3)
                    for k in range(KT):
                        nc.tensor.matmul(
                            ps1[:],
                            w1t[:, k * 128:(k + 1) * 128],
                            xt_t[k][:, s_off:s_off + s_len],
                            start=(k == 0), stop=(k == KT - 1),
                        )
                    gt = gtp.tile([128, 512], BF16, name=f"gt_{s_off}_{f}",
                                  tag=f"gt{f}")
                    nc.scalar.activation(
                        gt[:, :s_len], ps1[:],
                        mybir.ActivationFunctionType.Gelu,
                        bias=b1t[:, f:f + 1],
                    )
                    gts.append(gt)

                # prefetch next slice's xt columns while this slice computes
                # (emitted after MM1's w1 chunks so it doesn't queue ahead of
                # the critical first weight chunk at startup)
                if si + 1 < len(slices):
                    n_off, n_len = slices[si + 1]
                    for k in range(KT):
                        nc.sync.dma_start(
                            xt_t[k][:, n_off:n_off + n_len],
                            xt_d[k * 128:(k + 1) * 128, n_off:n_off + n_len])

                if si == 0:
                    # w2/b2t/wb fetched while the PE runs MM1(slice 0)
                    for f in range(FT):
                        nc.sync.dma_start(w2_t[f][:],
                                          w2_d[f * 128:(f + 1) * 128, :])
                    nc.sync.dma_start(b2t[:], b2t_d[:])
                    nc.sync.dma_start(wb[:], wb_d[:])

                # ---- MM2 (transposed out: YT[h, tokens], exact token width)
                #      + bias + combine-weight scale + store.
                # Early slices: h-groups of 4 with f outer, so w2[f] tiles
                # are consumed progressively while the 8MB w2 stream is still
                # in flight (4 PSUM banks). Final slice: h outer with f inner,
                # so only one drain chain trails the last matmul ----
                last = si == len(slices) - 1
                hg_size = 1 if last else 4
                for hg in range(0, H // 128, hg_size):
                    pss = [psp.tile([128, s_len], F32,
                                    name=f"ps2_{s_off}_{hg + j}", tag="ps2",
                                    bufs=5)
                           for j in range(hg_size)]
                    for f in range(FT):
                        for j in range(hg_size):
                            h = hg + j
                            nc.tensor.matmul(
                                pss[j][:],
                                w2_t[f][:, h * 128:(h + 1) * 128],
                                gts[f][:, :s_len],
                                start=(f == 0), stop=(f == FT - 1),
                            )
                    for j in range(hg_size):
                        h = hg + j
                        y_t = yst.tile([128, 512], F32, name=f"y_{s_off}_{h}",
                                       tag="y")
                        nc.vector.tensor_scalar_add(y_t[:, :s_len], pss[j][:],
                                                    b2t[:, h:h + 1])
                        nc.vector.tensor_mul(y_t[:, :s_len], y_t[:, :s_len],
                                             wb[:, s_off:s_off + s_len])
                        nc.sync.dma_start(
                            yt_d[h * 128:(h + 1) * 128, s_off:s_off + s_len],
                            y_t[:, :s_len])

    nc.compile()
    return nc


def kernel(x, gate_w, w1, b1, w2, b2):
    global LAST_RESULTS
    x = np.asarray(x, np.float32)
    gate_w = np.asarray(gate_w, np.float32)
    w1 = np.asarray(w1, np.float32)
    b1 = np.asarray(b1, np.float32)
    w2 = np.asarray(w2, np.float32)
    b2 = np.asarray(b2, np.float32)

    B, S, _ = x.shape
    xt = x.reshape(T, H)

    # ---- gate: softmax over experts, top-2, renormalize (host, fp32) ----
    logits = xt @ gate_w                                   # [T, E]
    logits = logits - logits.max(axis=1, keepdims=True)
    ex = np.exp(logits)
    rw = ex / ex.sum(axis=1, keepdims=True)                # routing_weights
    sel = np.argsort(-rw, axis=1, kind="stable")[:, :TOP_K]  # ties: low idx first
    tw = np.take_along_axis(rw, sel, axis=1)
    tw = tw / tw.sum(axis=1, keepdims=True)

    # ---- dispatch: token lists per expert ----
    idx_e, w_e = [], []
    for e in range(E):
        m0 = sel[:, 0] == e
        m1 = sel[:, 1] == e
        idx_e.append(np.concatenate([np.nonzero(m0)[0], np.nonzero(m1)[0]]))
        w_e.append(np.concatenate([tw[m0, 0], tw[m1, 1]]).astype(np.float32))
    # SBUF budget caps the per-core token capacity; if routing is ever
    # imbalanced enough to exceed it, split dispatch into multiple rounds
    C_SAFE = 2304
    rounds = max(1, -(-max(len(i) for i in idx_e) // C_SAFE))

    # per-expert weight tensors are round-invariant
    wt_maps = []
    for e in range(E):
        # w1c[f*128+p, k*128+j] = w1[e, k*128+p, f*128+j]
        w1c = np.ascontiguousarray(
            w1[e].astype(ml_dtypes.bfloat16)
            .reshape(KT, 128, FT, 128).transpose(2, 1, 0, 3).reshape(F, 1024)
        )
        wt_maps.append({
            "w1c": w1c,
            "w2": np.ascontiguousarray(w2[e].astype(ml_dtypes.bfloat16)),
            "b1t": np.ascontiguousarray(b1[e].reshape(FT, 128).T.astype(np.float32)),
            "b2t": np.ascontiguousarray(b2[e].reshape(H // 128, 128).T.astype(np.float32)),
        })

    xt_bf = xt.astype(ml_dtypes.bfloat16)
    out = np.zeros((T, H), np.float32)
    for r in range(rounds):
        idx_r = [i[r::rounds] for i in idx_e]
        w_r = [w[r::rounds] for w in w_e]
        counts = [len(i) for i in idx_r]
        C = max(512, -(-max(counts) // 128) * 128)
        C_x = min(C, -(-max(counts) // 2) * 2)   # exact MM1 width, 2-aligned

        key = (C, C_x)
        nc = _PROGRAM_CACHE.get(key)
        if nc is None:
            nc = build_moe_program(C, C_x)
            _PROGRAM_CACHE[key] = nc

        in_maps = []
        for e in range(E):
            n = counts[e]
            xe = np.zeros((C, H), ml_dtypes.bfloat16)
            xe[:n] = xt_bf[idx_r[e]]
            wpad = np.zeros(C, np.float32)
            wpad[:n] = w_r[e]
            in_maps.append({
                "xt": np.ascontiguousarray(xe.T),
                "wb": np.ascontiguousarray(np.broadcast_to(wpad, (128, C))),
                **wt_maps[e],
            })

        res = None
        for attempt in range(3):
            try:
                res = run_bass_kernel_spmd(nc, in_maps, core_ids=list(range(E)))
                break
            except Exception:
                # transient NRT_EXEC_UNIT_UNRECOVERABLE has been observed on
                # a fresh device; retry before giving up
                if attempt == 2:
                    raise
                time.sleep(2.0)
        LAST_RESULTS = res

        # ---- combine: each token gets exactly two expert contributions ----
        for e in range(E):
            n = counts[e]
            if n:
                out[idx_r[e]] += res.results[e]["yt"][:, :n].T

    return out.reshape(B, S, H), rw
